# Initial kernel scaffold
#
"""Trainium2 Bass kernel for CustomAttentionClassifier.

Model (see reference): x = emb[ids] + pe; Q/K/V = x@W + b;
attn = softmax(QK^T/16); pooled = mean_s(attn @ V); logits = relu(pooled@Wc1+bc1)@Wc2+bc2.

Sharding: data-parallel over batch, B=64 -> 8 cores x 8 batches.

Key restructuring (all computed per core):
- Host precomputes pQ = pe@Wq+bq (exact, fp64) etc., so the device only needs
  the embedding-dependent parts:  Q^T = Wq^T e^T + pQ^T  with e^T gathered
  directly in transposed layout by dma_gather(transpose=True) on a bf16 table.
- V is augmented with a ones column so the context matmul
  ctx'[s, 0:256]=sum_t exp[t,s] V[t,:], ctx'[s,256]=sum_t exp[t,s] also
  produces the softmax normalizers as a per-partition column.
- pooled^T[d] = sum_s ctx[s,d] * (1/(512*sums[s])) via N=1 matmuls, which both
  normalizes the softmax and performs the mean over s.
- bv needs no special handling: it is folded into pV.
"""

import numpy as np
import ml_dtypes

import concourse.bass as bass
import concourse.tile as tile
from concourse import bacc, mybir
from concourse.bass_utils import run_bass_kernel_spmd

V, D, S, B = 30522, 256, 512, 64
DP = D + 2  # fp32r matmuls need even free sizes; col D = softmax-sum ones, D+1 pad
HID, NCLS = 128, 16
NCORES = 8
BL = B // NCORES          # 8 batches per core
T = BL * S                # 4096 tokens per core
TCH = T // 128            # 32 token chunks
SCH = S // 128            # 4 s/t chunks per batch

f32 = mybir.dt.float32
f32r = mybir.dt.float32r
bf16 = mybir.dt.bfloat16
i16 = mybir.dt.int16

# knobs
import os as _os
CTX_DTYPE = _os.environ.get("CTX_DTYPE", "f32r")  # context matmul: f32r | f32 | bf16
STAGE = int(_os.environ.get("STAGE", "7"))  # debug truncation: 1=QK 2=+V 3=+exp 4=+ctx 5=+rcol 6=+pooled 7=full


def _pos_encoding():
    pos = np.arange(S)[:, None].astype(np.float64)
    div = np.exp(np.arange(0, D, 2).astype(np.float64) * (-np.log(10000.0) / D))
    pe = np.zeros((S, D), dtype=np.float64)
    pe[:, 0::2] = np.sin(pos * div)
    pe[:, 1::2] = np.cos(pos * div)
    # match the reference, which builds pe in float32
    return pe.astype(np.float32)


def build_module():
    nc = bacc.Bacc("TRN2", target_bir_lowering=False, debug=False)

    emb_d = nc.dram_tensor("emb16", [V, D], bf16, kind="ExternalInput")
    ids_d = nc.dram_tensor("ids16", [128, T // 16], i16, kind="ExternalInput")
    wq_d = nc.dram_tensor("wq16", [128, 2, D], bf16, kind="ExternalInput")
    wk_d = nc.dram_tensor("wk16", [128, 2, D], bf16, kind="ExternalInput")
    # wv/pv carry the augmentation column: wv[:, :, D] = 0, pv[:, :, D] = 1
    wv_d = nc.dram_tensor("wv16", [128, 2, DP], bf16, kind="ExternalInput")
    pqt_d = nc.dram_tensor("pqt", [128, 2, S], f32, kind="ExternalInput")
    pkt_d = nc.dram_tensor("pkt", [128, 2, S], f32, kind="ExternalInput")
    pv_d = nc.dram_tensor("pv", [128, SCH, DP], f32, kind="ExternalInput")
    wc1_d = nc.dram_tensor("wc1", [128, 2, HID], f32, kind="ExternalInput")
    bc1_d = nc.dram_tensor("bc1c", [128, 1], f32, kind="ExternalInput")
    wc2_d = nc.dram_tensor("wc2", [128, NCLS], f32, kind="ExternalInput")
    bc2_d = nc.dram_tensor("bc2c", [16, 1], f32, kind="ExternalInput")
    out_d = nc.dram_tensor("lgt", [NCLS, BL], f32, kind="ExternalOutput")

    ADD = mybir.AluOpType.add
    EXP = mybir.ActivationFunctionType.Exp
    RELU = mybir.ActivationFunctionType.Relu

    with tile.TileContext(nc) as tc:
        with (
            tc.tile_pool(name="const", bufs=1) as cp,
            tc.tile_pool(name="work", bufs=3) as wp,
            tc.tile_pool(name="psA", bufs=2, space="PSUM") as psA,
            tc.tile_pool(name="psB", bufs=4, space="PSUM") as psB,
        ):
            ids_s = cp.tile([128, T // 16], i16)
            wq_s = cp.tile([128, 2, D], bf16, tag="wq")
            wk_s = cp.tile([128, 2, D], bf16, tag="wk")
            wv_s = cp.tile([128, 2, DP], bf16, tag="wv")
            pqt_s = cp.tile([128, 2, S], f32, tag="pqt")
            pkt_s = cp.tile([128, 2, S], f32, tag="pkt")
            pv_s = cp.tile([128, SCH, DP], f32, tag="pv")
            wc1_s = cp.tile([128, 2, HID], f32, tag="wc1")
            bc1_s = cp.tile([128, 1], f32, tag="bc1")
            wc2_s = cp.tile([128, NCLS], f32, tag="wc2")
            bc2_s = cp.tile([16, 1], f32, tag="bc2")

            vdt = {"f32r": f32r, "f32": f32, "bf16": bf16}[CTX_DTYPE]
            # one gather chunk per batch: the SWDGE descriptor ring can't
            # hold 4096 descriptors in one instruction (device crash)
            eTs = [
                cp.tile([128, 2, S], bf16, tag=f"eT{n}", name=f"eT{n}")
                for n in range(BL)
            ]
            qT = cp.tile([128, 2, T], bf16, tag="qT")
            kT = cp.tile([128, 2, T], bf16, tag="kT")
            vS = cp.tile([128, TCH, DP], vdt, tag="vS")
            pooledT = cp.tile([128, 2, BL], f32, tag="pooledT")
            invS = cp.tile([128, SCH], f32, tag="invS")
            hT = cp.tile([128, BL], f32, tag="hT")
            lgT = cp.tile([16, BL], f32, tag="lgT")

            nc.sync.dma_start(ids_s[:], ids_d.ap())
            nc.sync.dma_start(wq_s[:], wq_d.ap())
            nc.sync.dma_start(wk_s[:], wk_d.ap())
            nc.sync.dma_start(wv_s[:], wv_d.ap())
            nc.sync.dma_start(pqt_s[:], pqt_d.ap())
            nc.sync.dma_start(pkt_s[:], pkt_d.ap())
            nc.sync.dma_start(pv_s[:], pv_d.ap())
            nc.sync.dma_start(wc1_s[:], wc1_d.ap())
            nc.sync.dma_start(bc1_s[:], bc1_d.ap())
            nc.sync.dma_start(wc2_s[:], wc2_d.ap())
            nc.sync.dma_start(bc2_s[:], bc2_d.ap())

            nc.vector.memset(invS[:], 1.0 / S)

            # gather e^T per batch: eTs[n][p, c, s] = emb16[ids[n*S+s], c*128+p]
            for n in range(BL):
                nc.gpsimd.dma_gather(
                    out_ap=eTs[n][:],
                    in_ap=emb_d.ap(),
                    idxs_ap=ids_s[:, n * (S // 16):(n + 1) * (S // 16)],
                    num_idxs=S,
                    num_idxs_reg=S,
                    elem_size=D,
                    transpose=True,
                )

            if STAGE < 7:
                nc.vector.memset(lgT[:], 0.0)

            # ---- Q^T, K^T ----
            for w_s, pT_s, oT in (((wq_s, pqt_s, qT), (wk_s, pkt_s, kT)) if STAGE >= 1 else ()):
                for m in range(2):
                    for n in range(BL):
                        ps = psB.tile([128, S], f32, tag="B")
                        for k in range(2):
                            nc.tensor.matmul(
                                ps[:, 0:S],
                                lhsT=w_s[:, k, m * 128:(m + 1) * 128],
                                rhs=eTs[n][:, k, :],
                                start=(k == 0),
                                stop=(k == 1),
                            )
                        nc.vector.tensor_tensor(
                            out=oT[:, m, n * S:(n + 1) * S],
                            in0=ps[:, 0:S],
                            in1=pT_s[:, m, :],
                            op=ADD,
                        )

            # ---- V (augmented) ----
            for c in range(TCH if STAGE >= 2 else 0):
                ps = psB.tile([128, DP], f32, tag="B")
                for k in range(2):
                    nc.tensor.matmul(
                        ps[:],
                        lhsT=eTs[c // SCH][:, k, (c % SCH) * 128:(c % SCH + 1) * 128],
                        rhs=wv_s[:, k, :],
                        start=(k == 0),
                        stop=(k == 1),
                    )
                nc.vector.tensor_tensor(
                    out=vS[:, c, :],
                    in0=ps[:],
                    in1=pv_s[:, c % SCH, :],
                    op=ADD,
                )

            # ---- attention + pooling, per batch ----
            for b in range(BL if STAGE >= 3 else 0):
                expT = wp.tile([128, SCH, S], vdt, tag="expT")
                for w in range(2):  # two waves of two t-chunks
                    ps = psA.tile([128, 2, S], f32, tag="A")
                    for i in range(2):
                        t = 2 * w + i
                        for m in range(2):
                            nc.tensor.matmul(
                                ps[:, i, :],
                                lhsT=kT[:, m, b * S + t * 128: b * S + (t + 1) * 128],
                                rhs=qT[:, m, b * S:(b + 1) * S],
                                start=(m == 0),
                                stop=(m == 1),
                            )
                    nc.scalar.activation(
                        out=expT[:, 2 * w:2 * w + 2, :],
                        in_=ps[:],
                        func=EXP,
                        scale=1.0 / 16.0,
                    )

                if STAGE < 4:
                    continue
                ctx_sb = wp.tile([128, SCH, DP], bf16, tag="ctx")
                for sc in range(SCH):
                    cps = psB.tile([128, DP], f32, tag="B")
                    for t in range(SCH):
                        nc.tensor.matmul(
                            cps[:],
                            lhsT=expT[:, t, sc * 128:(sc + 1) * 128],
                            rhs=vS[:, b * SCH + t, :],
                            start=(t == 0),
                            stop=(t == SCH - 1),
                        )
                    nc.scalar.copy(out=ctx_sb[:, sc, :], in_=cps[:])

                # rcol = (1/512) / sums  (bf16 for the pooled matmul)
                if STAGE < 5:
                    continue
                rc32 = wp.tile([128, SCH], f32, tag="rc32")
                rcol = wp.tile([128, SCH], bf16, tag="rcol")
                nc.vector.reciprocal(out=rc32[:], in_=ctx_sb[:, :, D])
                nc.vector.tensor_tensor(
                    out=rcol[:], in0=rc32[:], in1=invS[:], op=mybir.AluOpType.mult
                )

                if STAGE < 6:
                    continue
                for dch in range(2):
                    pps = psB.tile([128, DP], f32, tag="B")
                    for sc in range(SCH):
                        nc.tensor.matmul(
                            pps[:, 0:1],
                            lhsT=ctx_sb[:, sc, dch * 128:(dch + 1) * 128],
                            rhs=rcol[:, sc:sc + 1],
                            start=(sc == 0),
                            stop=(sc == SCH - 1),
                        )
                    nc.vector.tensor_copy(
                        out=pooledT[:, dch, b:b + 1], in_=pps[:, 0:1]
                    )

            # ---- classifier ----
            if STAGE >= 7:
                hps = psB.tile([128, DP], f32, tag="B")
                for k in range(2):
                    nc.tensor.matmul(
                        hps[:, 0:BL],
                        lhsT=wc1_s[:, k, :],
                        rhs=pooledT[:, k, :],
                        start=(k == 0),
                        stop=(k == 1),
                    )
                nc.scalar.activation(
                    out=hT[:], in_=hps[:, 0:BL], func=RELU, bias=bc1_s[:]
                )

                lps = psB.tile([128, DP], f32, tag="B")
                nc.tensor.matmul(
                    lps[0:NCLS, 0:BL], lhsT=wc2_s[:], rhs=hT[:], start=True, stop=True
                )
                nc.vector.tensor_tensor(
                    out=lgT[:],
                    in0=lps[0:NCLS, 0:BL],
                    in1=bc2_s[:].to_broadcast([NCLS, BL]),
                    op=ADD,
                )
            nc.sync.dma_start(out_d.ap(), lgT[:])

    nc.compile()
    return nc


def prepare_in_maps(input_ids, emb, Wq, bq, Wk, bk, Wv, bv, Wc1, bc1, Wc2, bc2):
    pe = _pos_encoding().astype(np.float64)
    pQ = (pe @ Wq.astype(np.float64) + bq.astype(np.float64)).astype(np.float32)
    pK = (pe @ Wk.astype(np.float64) + bk.astype(np.float64)).astype(np.float32)
    pV = (pe @ Wv.astype(np.float64) + bv.astype(np.float64)).astype(np.float32)

    emb16 = np.ascontiguousarray(emb.astype(ml_dtypes.bfloat16))

    def chunk_w(w):  # [D, D] -> [128, 2, D] bf16 with [p,k,j] = w[k*128+p, j]
        return np.ascontiguousarray(
            w.reshape(2, 128, D).transpose(1, 0, 2).astype(ml_dtypes.bfloat16)
        )

    wq16 = chunk_w(Wq)
    wk16 = chunk_w(Wk)
    wv16 = np.zeros((128, 2, DP), dtype=ml_dtypes.bfloat16)
    wv16[:, :, :D] = chunk_w(Wv)
    pV_aug = np.zeros((S, DP), dtype=np.float32)
    pV_aug[:, :D] = pV
    pV_aug[:, D] = 1.0

    def chunk_pT(p):  # [S, D] -> [128, 2, S] f32 with [p_,m,s] = p[s, m*128+p_]
        return np.ascontiguousarray(p.T.reshape(2, 128, S).transpose(1, 0, 2)).astype(
            np.float32
        )

    pqt = chunk_pT(pQ)
    pkt = chunk_pT(pK)
    pv_l = np.ascontiguousarray(
        pV_aug.reshape(SCH, 128, DP).transpose(1, 0, 2)
    ).astype(np.float32)
    wc1 = np.ascontiguousarray(
        Wc1.reshape(2, 128, HID).transpose(1, 0, 2).astype(np.float32)
    )
    bc1c = np.ascontiguousarray(bc1.reshape(HID, 1).astype(np.float32))
    wc2 = np.ascontiguousarray(Wc2.astype(np.float32))
    bc2c = np.ascontiguousarray(bc2.reshape(NCLS, 1).astype(np.float32))

    in_maps = []
    for c in range(NCORES):
        ids = input_ids[c * BL:(c + 1) * BL].reshape(-1).astype(np.int16)
        ids16 = np.tile(ids.reshape(T // 16, 16).T, (8, 1))  # [128, T//16]
        in_maps.append(
            dict(
                emb16=emb16,
                ids16=np.ascontiguousarray(ids16),
                wq16=wq16,
                wk16=wk16,
                wv16=wv16,
                pqt=pqt,
                pkt=pkt,
                pv=pv_l,
                wc1=wc1,
                bc1c=bc1c,
                wc2=wc2,
                bc2c=bc2c,
            )
        )
    return in_maps


_NC_CACHE = {}


def kernel(**inputs):
    inputs = {k: np.asarray(v) for k, v in inputs.items()}
    if "nc" not in _NC_CACHE:
        _NC_CACHE["nc"] = build_module()
    nc = _NC_CACHE["nc"]
    in_maps = prepare_in_maps(**inputs)
    res = run_bass_kernel_spmd(nc, in_maps, core_ids=list(range(NCORES)))
    out = np.empty((B, NCLS), dtype=np.float32)
    for c in range(NCORES):
        out[c * BL:(c + 1) * BL] = res.results[c]["lgt"].T
    return out



# revision 1
# speedup vs baseline: 1.0831x; 1.0831x over previous
"""Trainium2 Bass kernel for CustomAttentionClassifier.

Model (see reference): x = emb[ids] + pe; Q/K/V = x@W + b;
attn = softmax(QK^T/16); pooled = mean_s(attn @ V); logits = relu(pooled@Wc1+bc1)@Wc2+bc2.

Sharding: data-parallel over batch, B=64 -> 8 cores x 8 batches.

Key restructuring (all computed per core):
- Host precomputes pQ = pe@Wq+bq (exact, fp64) etc., so the device only needs
  the embedding-dependent parts:  Q^T = Wq^T e^T + pQ^T  with e^T gathered
  directly in transposed layout by dma_gather(transpose=True) on a bf16 table.
- V is augmented with a ones column so the context matmul
  ctx'[s, 0:256]=sum_t exp[t,s] V[t,:], ctx'[s,256]=sum_t exp[t,s] also
  produces the softmax normalizers as a per-partition column.
- pooled^T[d] = sum_s ctx[s,d] * (1/(512*sums[s])) via N=1 matmuls, which both
  normalizes the softmax and performs the mean over s.
- bv needs no special handling: it is folded into pV.
"""

import numpy as np
import ml_dtypes

import concourse.bass as bass
import concourse.tile as tile
from concourse import bacc, mybir
from concourse.bass_utils import run_bass_kernel_spmd

V, D, S, B = 30522, 256, 512, 64
DP = D + 2  # fp32r matmuls need even free sizes; col D = softmax-sum ones, D+1 pad
HID, NCLS = 128, 16
NCORES = 8
BL = B // NCORES          # 8 batches per core
T = BL * S                # 4096 tokens per core
TCH = T // 128            # 32 token chunks
SCH = S // 128            # 4 s/t chunks per batch

f32 = mybir.dt.float32
f32r = mybir.dt.float32r
bf16 = mybir.dt.bfloat16
i16 = mybir.dt.int16

# knobs
import os as _os
CTX_DTYPE = _os.environ.get("CTX_DTYPE", "f32r")  # context matmul: f32r | f32 | bf16
STAGE = int(_os.environ.get("STAGE", "7"))  # debug truncation: 1=QK 2=+V 3=+exp 4=+ctx 5=+rcol 6=+pooled 7=full


def _pos_encoding():
    pos = np.arange(S)[:, None].astype(np.float64)
    div = np.exp(np.arange(0, D, 2).astype(np.float64) * (-np.log(10000.0) / D))
    pe = np.zeros((S, D), dtype=np.float64)
    pe[:, 0::2] = np.sin(pos * div)
    pe[:, 1::2] = np.cos(pos * div)
    # match the reference, which builds pe in float32
    return pe.astype(np.float32)


def build_module():
    nc = bacc.Bacc("TRN2", target_bir_lowering=False, debug=False)

    emb_d = nc.dram_tensor("emb16", [V, D], bf16, kind="ExternalInput")
    ids_d = nc.dram_tensor("ids16", [128, T // 16], i16, kind="ExternalInput")
    wq_d = nc.dram_tensor("wq16", [128, 2, D], bf16, kind="ExternalInput")
    wk_d = nc.dram_tensor("wk16", [128, 2, D], bf16, kind="ExternalInput")
    # wv/pv carry the augmentation column: wv[:, :, D] = 0, pv[:, :, D] = 1
    wv_d = nc.dram_tensor("wv16", [128, 2, DP], bf16, kind="ExternalInput")
    pqt_d = nc.dram_tensor("pqt", [128, 2, S], f32, kind="ExternalInput")
    pkt_d = nc.dram_tensor("pkt", [128, 2, S], f32, kind="ExternalInput")
    pv_d = nc.dram_tensor("pv", [128, SCH, DP], f32, kind="ExternalInput")
    wc1_d = nc.dram_tensor("wc1", [128, 2, HID], f32, kind="ExternalInput")
    bc1_d = nc.dram_tensor("bc1c", [128, 1], f32, kind="ExternalInput")
    wc2_d = nc.dram_tensor("wc2", [128, NCLS], f32, kind="ExternalInput")
    bc2_d = nc.dram_tensor("bc2c", [16, 1], f32, kind="ExternalInput")
    out_d = nc.dram_tensor("lgt", [NCLS, BL], f32, kind="ExternalOutput")

    ADD = mybir.AluOpType.add
    EXP = mybir.ActivationFunctionType.Exp
    RELU = mybir.ActivationFunctionType.Relu

    with tile.TileContext(nc) as tc:
        with (
            tc.tile_pool(name="const", bufs=1) as cp,
            tc.tile_pool(name="work", bufs=3) as wp,
            tc.tile_pool(name="psA", bufs=2, space="PSUM") as psA,
            tc.tile_pool(name="psB", bufs=4, space="PSUM") as psB,
        ):
            ids_s = cp.tile([128, T // 16], i16)
            wq_s = cp.tile([128, 2, D], bf16, tag="wq")
            wk_s = cp.tile([128, 2, D], bf16, tag="wk")
            wv_s = cp.tile([128, 2, DP], bf16, tag="wv")
            pqt_s = cp.tile([128, 2, S], f32, tag="pqt")
            pkt_s = cp.tile([128, 2, S], f32, tag="pkt")
            pv_s = cp.tile([128, SCH, DP], f32, tag="pv")
            wc1_s = cp.tile([128, 2, HID], f32, tag="wc1")
            bc1_s = cp.tile([128, 1], f32, tag="bc1")
            wc2_s = cp.tile([128, NCLS], f32, tag="wc2")
            bc2_s = cp.tile([16, 1], f32, tag="bc2")

            vdt = {"f32r": f32r, "f32": f32, "bf16": bf16}[CTX_DTYPE]
            # one gather chunk per batch: the SWDGE descriptor ring can't
            # hold 4096 descriptors in one instruction (device crash)
            eTs = [
                cp.tile([128, 2, S], bf16, tag=f"eT{n}", name=f"eT{n}")
                for n in range(BL)
            ]
            qT = cp.tile([128, 2, T], bf16, tag="qT")
            kT = cp.tile([128, 2, T], bf16, tag="kT")
            vS = cp.tile([128, TCH, DP], vdt, tag="vS")
            pooledT = cp.tile([128, 2, BL], f32, tag="pooledT")
            invS = cp.tile([128, SCH], f32, tag="invS")
            hT = cp.tile([128, BL], f32, tag="hT")
            lgT = cp.tile([16, BL], f32, tag="lgT")

            nc.sync.dma_start(ids_s[:], ids_d.ap())
            nc.sync.dma_start(wq_s[:], wq_d.ap())
            nc.sync.dma_start(wk_s[:], wk_d.ap())
            nc.sync.dma_start(wv_s[:], wv_d.ap())
            nc.sync.dma_start(pqt_s[:], pqt_d.ap())
            nc.sync.dma_start(pkt_s[:], pkt_d.ap())
            nc.sync.dma_start(pv_s[:], pv_d.ap())
            nc.sync.dma_start(wc1_s[:], wc1_d.ap())
            nc.sync.dma_start(bc1_s[:], bc1_d.ap())
            nc.sync.dma_start(wc2_s[:], wc2_d.ap())
            nc.sync.dma_start(bc2_s[:], bc2_d.ap())

            nc.vector.memset(invS[:], 1.0 / S)

            # gather e^T per batch: eTs[n][p, c, s] = emb16[ids[n*S+s], c*128+p]
            for n in range(BL):
                nc.gpsimd.dma_gather(
                    out_ap=eTs[n][:],
                    in_ap=emb_d.ap(),
                    idxs_ap=ids_s[:, n * (S // 16):(n + 1) * (S // 16)],
                    num_idxs=S,
                    num_idxs_reg=S,
                    elem_size=D,
                    transpose=True,
                )

            if STAGE < 7:
                nc.vector.memset(lgT[:], 0.0)

            # ---- Q^T, K^T ----
            for w_s, pT_s, oT in (((wq_s, pqt_s, qT), (wk_s, pkt_s, kT)) if STAGE >= 1 else ()):
                for m in range(2):
                    for n in range(BL):
                        ps = psB.tile([128, S], f32, tag="B")
                        for k in range(2):
                            nc.tensor.matmul(
                                ps[:, 0:S],
                                lhsT=w_s[:, k, m * 128:(m + 1) * 128],
                                rhs=eTs[n][:, k, :],
                                start=(k == 0),
                                stop=(k == 1),
                            )
                        nc.vector.tensor_tensor(
                            out=oT[:, m, n * S:(n + 1) * S],
                            in0=ps[:, 0:S],
                            in1=pT_s[:, m, :],
                            op=ADD,
                        )

            # ---- V (augmented) ----
            for c in range(TCH if STAGE >= 2 else 0):
                ps = psB.tile([128, DP], f32, tag="B")
                for k in range(2):
                    nc.tensor.matmul(
                        ps[:],
                        lhsT=eTs[c // SCH][:, k, (c % SCH) * 128:(c % SCH + 1) * 128],
                        rhs=wv_s[:, k, :],
                        start=(k == 0),
                        stop=(k == 1),
                    )
                nc.vector.tensor_tensor(
                    out=vS[:, c, :],
                    in0=ps[:],
                    in1=pv_s[:, c % SCH, :],
                    op=ADD,
                )

            # ---- attention + pooling, per batch ----
            for b in range(BL if STAGE >= 3 else 0):
                expT = wp.tile([128, SCH, S], vdt, tag="expT")
                for w in range(2):  # two waves of two t-chunks
                    ps = psA.tile([128, 2, S], f32, tag="A")
                    for i in range(2):
                        t = 2 * w + i
                        for m in range(2):
                            nc.tensor.matmul(
                                ps[:, i, :],
                                lhsT=kT[:, m, b * S + t * 128: b * S + (t + 1) * 128],
                                rhs=qT[:, m, b * S:(b + 1) * S],
                                start=(m == 0),
                                stop=(m == 1),
                            )
                    nc.scalar.activation(
                        out=expT[:, 2 * w:2 * w + 2, :],
                        in_=ps[:],
                        func=EXP,
                        scale=1.0 / 16.0,
                    )

                if STAGE < 4:
                    continue
                ctx_sb = wp.tile([128, SCH, DP], bf16, tag="ctx")
                for sc in range(SCH):
                    cps = psB.tile([128, DP], f32, tag="B")
                    for t in range(SCH):
                        nc.tensor.matmul(
                            cps[:],
                            lhsT=expT[:, t, sc * 128:(sc + 1) * 128],
                            rhs=vS[:, b * SCH + t, :],
                            start=(t == 0),
                            stop=(t == SCH - 1),
                        )
                    nc.scalar.copy(out=ctx_sb[:, sc, :], in_=cps[:])

                # rcol = (1/512) / sums  (bf16 for the pooled matmul)
                if STAGE < 5:
                    continue
                rc32 = wp.tile([128, SCH], f32, tag="rc32")
                rcol = wp.tile([128, SCH], bf16, tag="rcol")
                nc.vector.reciprocal(out=rc32[:], in_=ctx_sb[:, :, D])
                nc.vector.tensor_tensor(
                    out=rcol[:], in0=rc32[:], in1=invS[:], op=mybir.AluOpType.mult
                )

                if STAGE < 6:
                    continue
                for dch in range(2):
                    pps = psB.tile([128, DP], f32, tag="B")
                    for sc in range(SCH):
                        nc.tensor.matmul(
                            pps[:, 0:1],
                            lhsT=ctx_sb[:, sc, dch * 128:(dch + 1) * 128],
                            rhs=rcol[:, sc:sc + 1],
                            start=(sc == 0),
                            stop=(sc == SCH - 1),
                        )
                    nc.vector.tensor_copy(
                        out=pooledT[:, dch, b:b + 1], in_=pps[:, 0:1]
                    )

            # ---- classifier ----
            if STAGE >= 7:
                hps = psB.tile([128, DP], f32, tag="B")
                for k in range(2):
                    nc.tensor.matmul(
                        hps[:, 0:BL],
                        lhsT=wc1_s[:, k, :],
                        rhs=pooledT[:, k, :],
                        start=(k == 0),
                        stop=(k == 1),
                    )
                nc.scalar.activation(
                    out=hT[:], in_=hps[:, 0:BL], func=RELU, bias=bc1_s[:]
                )

                lps = psB.tile([128, DP], f32, tag="B")
                nc.tensor.matmul(
                    lps[0:NCLS, 0:BL], lhsT=wc2_s[:], rhs=hT[:], start=True, stop=True
                )
                nc.vector.tensor_tensor(
                    out=lgT[:],
                    in0=lps[0:NCLS, 0:BL],
                    in1=bc2_s[:].to_broadcast([NCLS, BL]),
                    op=ADD,
                )
            nc.sync.dma_start(out_d.ap(), lgT[:])

    nc.compile()
    return nc


def prepare_in_maps(input_ids, emb, Wq, bq, Wk, bk, Wv, bv, Wc1, bc1, Wc2, bc2):
    pe = _pos_encoding().astype(np.float64)
    pQ = (pe @ Wq.astype(np.float64) + bq.astype(np.float64)).astype(np.float32)
    pK = (pe @ Wk.astype(np.float64) + bk.astype(np.float64)).astype(np.float32)
    pV = (pe @ Wv.astype(np.float64) + bv.astype(np.float64)).astype(np.float32)

    emb16 = np.ascontiguousarray(emb.astype(ml_dtypes.bfloat16))

    def chunk_w(w):  # [D, D] -> [128, 2, D] bf16 with [p,k,j] = w[k*128+p, j]
        return np.ascontiguousarray(
            w.reshape(2, 128, D).transpose(1, 0, 2).astype(ml_dtypes.bfloat16)
        )

    wq16 = chunk_w(Wq)
    wk16 = chunk_w(Wk)
    wv16 = np.zeros((128, 2, DP), dtype=ml_dtypes.bfloat16)
    wv16[:, :, :D] = chunk_w(Wv)
    pV_aug = np.zeros((S, DP), dtype=np.float32)
    pV_aug[:, :D] = pV
    pV_aug[:, D] = 1.0

    def chunk_pT(p):  # [S, D] -> [128, 2, S] f32 with [p_,m,s] = p[s, m*128+p_]
        return np.ascontiguousarray(p.T.reshape(2, 128, S).transpose(1, 0, 2)).astype(
            np.float32
        )

    pqt = chunk_pT(pQ)
    pkt = chunk_pT(pK)
    pv_l = np.ascontiguousarray(
        pV_aug.reshape(SCH, 128, DP).transpose(1, 0, 2)
    ).astype(np.float32)
    wc1 = np.ascontiguousarray(
        Wc1.reshape(2, 128, HID).transpose(1, 0, 2).astype(np.float32)
    )
    bc1c = np.ascontiguousarray(bc1.reshape(HID, 1).astype(np.float32))
    wc2 = np.ascontiguousarray(Wc2.astype(np.float32))
    bc2c = np.ascontiguousarray(bc2.reshape(NCLS, 1).astype(np.float32))

    in_maps = []
    for c in range(NCORES):
        ids = input_ids[c * BL:(c + 1) * BL].reshape(-1).astype(np.int16)
        ids16 = np.tile(ids.reshape(T // 16, 16).T, (8, 1))  # [128, T//16]
        in_maps.append(
            dict(
                emb16=emb16,
                ids16=np.ascontiguousarray(ids16),
                wq16=wq16,
                wk16=wk16,
                wv16=wv16,
                pqt=pqt,
                pkt=pkt,
                pv=pv_l,
                wc1=wc1,
                bc1c=bc1c,
                wc2=wc2,
                bc2c=bc2c,
            )
        )
    return in_maps


_NC_CACHE = {}


def kernel(**inputs):
    inputs = {k: np.asarray(v) for k, v in inputs.items()}
    if "nc" not in _NC_CACHE:
        _NC_CACHE["nc"] = build_module()
    nc = _NC_CACHE["nc"]
    in_maps = prepare_in_maps(**inputs)
    res = run_bass_kernel_spmd(nc, in_maps, core_ids=list(range(NCORES)))
    out = np.empty((B, NCLS), dtype=np.float32)
    for c in range(NCORES):
        out[c * BL:(c + 1) * BL] = res.results[c]["lgt"].T
    return out

